# revision 16
# baseline (speedup 1.0000x reference)
"""BERT embedding (token/type/position gather + LayerNorm) on 8 Trainium2 cores.

Sharding: data-parallel over batch — core c handles sequences [4c, 4c+4),
i.e. 2048 tokens. Each core holds an augmented embedding table
[token_W; token_W + (type_W[1]-type_W[0])] and gathers row (id + t*V) with
indirect DMA, which folds the token-type embedding into the gather.
type_W[0] is folded into the position table on the host. The position row
is added via a GpSimd prefill + accumulate-DMA (or DVE add, configurable).
LayerNorm runs per 128-token tile with bn_stats/bn_aggr; the final
(x-mean)*rstd is applied on the scalar engine as Copy(x*rstd + (-mean*rstd)).
"""
import numpy as np

import concourse.bacc as bacc
import concourse.bass as bass
import concourse.tile as tile
from concourse import mybir
from concourse.bass_utils import run_bass_kernel_spmd

P = 128
N_CORES = 8
B, S, V, H, T = 32, 512, 30522, 1024, 2
EPS = 1e-5
B_PER_CORE = B // N_CORES       # 4 sequences per core
N_TOK = B_PER_CORE * S          # 2048 tokens per core
NT = N_TOK // P                 # 16 token tiles per core
Q = S // P                      # 4 position quarters

F32 = mybir.dt.float32
I32 = mybir.dt.int32

# POS_MODE: how pos_comb gets added to the gathered row
#   "dma_add"  — GpSimd tensor_copy prefill, gather DMA accumulates (CCE add)
#   "dve_add"  — plain gather, DVE tensor_add afterwards
POS_MODE = "dve_add"
# NORM_MODE: "act" = Copy(x*rstd + nb) on scalar engine; "dve" = tensor_scalar
NORM_MODE = "act"
BUFS_TE = 16
BUFS_O = 6
POS_ADD_SPLIT = False

_cache: dict = {}


def _build(apply_ln: bool):
    nc = bacc.Bacc(None, target_bir_lowering=False)
    aug_w = nc.declare_dram_parameter("aug_w", [2 * V, H], F32, isOutput=False)
    ids = nc.declare_dram_parameter("ids", [P, NT], I32, isOutput=False)
    pos_c = nc.declare_dram_parameter("pos_c", [P, Q, H], F32, isOutput=False)
    if apply_ln:
        lnw = nc.declare_dram_parameter("lnw", [1, H], F32, isOutput=False)
        lnb = nc.declare_dram_parameter("lnb", [1, H], F32, isOutput=False)
    out_d = nc.declare_dram_parameter("out", [N_TOK, H], F32, isOutput=True)

    with tile.TileContext(nc) as tc:
        with (
            tc.tile_pool(name="singles", bufs=1) as singles,
            tc.tile_pool(name="te_p", bufs=BUFS_TE) as te_p,
            tc.tile_pool(name="o_p", bufs=BUFS_O) as o_p,
            tc.tile_pool(name="stats", bufs=8) as stats_p,
        ):
            ids_sb = singles.tile([P, NT], I32)
            nc.sync.dma_start(out=ids_sb[:], in_=ids[:])
            pos_sb = singles.tile([P, Q, H], F32)
            nc.sync.dma_start(out=pos_sb[:, 0, :], in_=pos_c[:, 0, :])
            eps_sb = singles.tile([P, 1], F32)
            nc.vector.memset(eps_sb[:], EPS)
            if apply_ln:
                lnw_sb = singles.tile([P, H], F32)
                nc.gpsimd.dma_start(out=lnw_sb[:], in_=lnw[:].to_broadcast([P, H]))
                lnb_sb = singles.tile([P, H], F32)
                nc.gpsimd.dma_start(out=lnb_sb[:], in_=lnb[:].to_broadcast([P, H]))

            for j in range(NT):
                te = te_p.tile([P, H], F32, tag="te")
                off = bass.IndirectOffsetOnAxis(ap=ids_sb[:, j:j + 1], axis=0)
                if 1 <= j <= Q - 1:
                    nc.sync.dma_start(out=pos_sb[:, j, :], in_=pos_c[:, j, :])
                if POS_MODE == "dma_add":
                    nc.gpsimd.tensor_copy(out=te[:], in_=pos_sb[:, j % Q, :])
                    nc.gpsimd.indirect_dma_start(
                        out=te[:], out_offset=None, in_=aug_w[:], in_offset=off,
                        compute_op=mybir.AluOpType.add,
                    )
                else:
                    nc.gpsimd.indirect_dma_start(
                        out=te[:], out_offset=None, in_=aug_w[:], in_offset=off,
                    )
                    add_eng = nc.gpsimd if (POS_ADD_SPLIT and j % 2) else nc.vector
                    add_eng.tensor_add(out=te[:], in0=te[:],
                                       in1=pos_sb[:, j % Q, :])

                stats = stats_p.tile([P, 2, 6], F32, tag="st")
                nc.vector.bn_stats(out=stats[:, 0, :], in_=te[:, 0:512])
                nc.vector.bn_stats(out=stats[:, 1, :], in_=te[:, 512:H])
                mv = stats_p.tile([P, 2], F32, tag="mv")
                nc.vector.bn_aggr(out=mv[:], in_=stats[:])

                std = stats_p.tile([P, 1], F32, tag="sd")
                nc.scalar.activation(
                    out=std[:], in_=mv[:, 1:2],
                    func=mybir.ActivationFunctionType.Sqrt,
                    bias=eps_sb[:], scale=1.0,
                )
                nc.vector.reciprocal(out=std[:], in_=std[:])

                o = o_p.tile([P, H], F32, tag="o")
                if NORM_MODE == "act":
                    nb = stats_p.tile([P, 1], F32, tag="nb")
                    nc.gpsimd.tensor_scalar(
                        out=nb[:], in0=mv[:, 0:1], scalar1=std[:], scalar2=-1.0,
                        op0=mybir.AluOpType.mult, op1=mybir.AluOpType.mult,
                    )
                    nc.scalar.activation(
                        out=o[:], in_=te[:],
                        func=mybir.ActivationFunctionType.Identity,
                        bias=nb[:], scale=std[:],
                    )
                else:
                    nc.vector.tensor_scalar(
                        out=o[:], in0=te[:], scalar1=mv[:, 0:1], scalar2=std[:],
                        op0=mybir.AluOpType.subtract, op1=mybir.AluOpType.mult,
                    )
                if apply_ln:
                    nc.vector.tensor_mul(out=o[:], in0=o[:], in1=lnw_sb[:])
                    nc.vector.tensor_add(out=o[:], in0=o[:], in1=lnb_sb[:])
                nc.scalar.dma_start(out=out_d[j * P:(j + 1) * P, :], in_=o[:])
    nc.finalize()
    return nc


def _prepare_inputs(input_ids, token_type_ids, token_W, pos_W, type_W,
                    ln_w, ln_b, apply_ln):
    ids_np = np.asarray(input_ids).reshape(B, S)
    tt_np = np.asarray(token_type_ids).reshape(B, S)
    tok = np.asarray(token_W, dtype=np.float32)
    pos = np.asarray(pos_W, dtype=np.float32)
    typ = np.asarray(type_W, dtype=np.float32)

    aug = np.empty((2 * V, H), dtype=np.float32)
    aug[:V] = tok
    np.add(tok, typ[1] - typ[0], out=aug[V:])

    pos_comb = pos[:S] + typ[0]                                     # [S, H]
    pos_hw = np.ascontiguousarray(pos_comb.reshape(Q, P, H).transpose(1, 0, 2))

    ids_aug = (ids_np.astype(np.int64) + tt_np.astype(np.int64) * V).astype(np.int32)

    in_maps = []
    for c in range(N_CORES):
        idc = ids_aug[c * B_PER_CORE:(c + 1) * B_PER_CORE].reshape(N_TOK)
        m = {
            "aug_w": aug,
            "ids": np.ascontiguousarray(idc.reshape(NT, P).T),
            "pos_c": pos_hw,
        }
        if apply_ln:
            m["lnw"] = np.ascontiguousarray(
                np.asarray(ln_w, dtype=np.float32).reshape(1, H))
            m["lnb"] = np.ascontiguousarray(
                np.asarray(ln_b, dtype=np.float32).reshape(1, H))
        in_maps.append(m)
    return in_maps


def _run(input_ids, token_type_ids, token_W, pos_W, type_W, ln_w, ln_b,
         trace=False):
    lnw = np.asarray(ln_w, dtype=np.float32).reshape(-1)
    lnb = np.asarray(ln_b, dtype=np.float32).reshape(-1)
    apply_ln = not (np.all(lnw == 1.0) and np.all(lnb == 0.0))

    nc = _cache.get(apply_ln)
    if nc is None:
        nc = _cache.setdefault(apply_ln, _build(apply_ln))
    in_maps = _prepare_inputs(input_ids, token_type_ids, token_W, pos_W,
                              type_W, ln_w, ln_b, apply_ln)
    res = run_bass_kernel_spmd(nc, in_maps, list(range(N_CORES)), trace=trace)
    out = np.concatenate(
        [res.results[c]["out"].reshape(B_PER_CORE, S, H) for c in range(N_CORES)],
        axis=0,
    )
    return out, res


def kernel(input_ids, token_type_ids, token_W, pos_W, type_W, ln_w, ln_b):
    out, _ = _run(input_ids, token_type_ids, token_W, pos_W, type_W,
                  ln_w, ln_b, trace=False)
    return out


# revision 17
# speedup vs baseline: 1.1570x; 1.1570x over previous
"""BERT embedding (token/type/position gather + LayerNorm) on 8 Trainium2 cores.

Sharding: data-parallel over batch — core c handles sequences [4c, 4c+4),
i.e. 2048 tokens. Each core holds an augmented embedding table
[token_W; token_W + (type_W[1]-type_W[0])] and gathers row (id + t*V) with
indirect DMA, which folds the token-type embedding into the gather.
type_W[0] is folded into the position table on the host. The position row
is added via a GpSimd prefill + accumulate-DMA (or DVE add, configurable).
LayerNorm runs per 128-token tile with bn_stats/bn_aggr; the final
(x-mean)*rstd is applied on the scalar engine as Copy(x*rstd + (-mean*rstd)).
"""
import numpy as np

import concourse.bacc as bacc
import concourse.bass as bass
import concourse.tile as tile
from concourse import mybir
from concourse.bass_utils import run_bass_kernel_spmd

P = 128
N_CORES = 8
B, S, V, H, T = 32, 512, 30522, 1024, 2
EPS = 1e-5
B_PER_CORE = B // N_CORES       # 4 sequences per core
N_TOK = B_PER_CORE * S          # 2048 tokens per core
NT = N_TOK // P                 # 16 token tiles per core
Q = S // P                      # 4 position quarters

F32 = mybir.dt.float32
I32 = mybir.dt.int32

# POS_MODE: how pos_comb gets added to the gathered row
#   "dma_add"  — GpSimd tensor_copy prefill, gather DMA accumulates (CCE add)
#   "dve_add"  — plain gather, DVE tensor_add afterwards
POS_MODE = "dve_add"
# NORM_MODE: "act" = Copy(x*rstd + nb) on scalar engine; "dve" = tensor_scalar
NORM_MODE = "act"
BUFS_TE = 16
BUFS_O = 6
POS_ADD_SPLIT = False

_cache: dict = {}


def _build(apply_ln: bool):
    nc = bacc.Bacc(None, target_bir_lowering=False)
    aug_w = nc.declare_dram_parameter("aug_w", [2 * V, H], F32, isOutput=False)
    ids = nc.declare_dram_parameter("ids", [P, NT], I32, isOutput=False)
    pos_c = nc.declare_dram_parameter("pos_c", [P, Q, H], F32, isOutput=False)
    if apply_ln:
        lnw = nc.declare_dram_parameter("lnw", [1, H], F32, isOutput=False)
        lnb = nc.declare_dram_parameter("lnb", [1, H], F32, isOutput=False)
    out_d = nc.declare_dram_parameter("out", [N_TOK, H], F32, isOutput=True)

    with tile.TileContext(nc) as tc:
        with (
            tc.tile_pool(name="singles", bufs=1) as singles,
            tc.tile_pool(name="te_p", bufs=BUFS_TE) as te_p,
            tc.tile_pool(name="o_p", bufs=BUFS_O) as o_p,
            tc.tile_pool(name="stats", bufs=8) as stats_p,
        ):
            ids_sb = singles.tile([P, NT], I32)
            nc.sync.dma_start(out=ids_sb[:], in_=ids[:])
            pos_sb = singles.tile([P, Q, H], F32)
            nc.sync.dma_start(out=pos_sb[:, 0, :], in_=pos_c[:, 0, :])
            eps_sb = singles.tile([P, 1], F32)
            nc.vector.memset(eps_sb[:], EPS)
            if apply_ln:
                lnw_sb = singles.tile([P, H], F32)
                nc.gpsimd.dma_start(out=lnw_sb[:], in_=lnw[:].to_broadcast([P, H]))
                lnb_sb = singles.tile([P, H], F32)
                nc.gpsimd.dma_start(out=lnb_sb[:], in_=lnb[:].to_broadcast([P, H]))

            for j in range(NT):
                te = te_p.tile([P, H], F32, tag="te")
                off = bass.IndirectOffsetOnAxis(ap=ids_sb[:, j:j + 1], axis=0)
                if 1 <= j <= Q - 1:
                    nc.sync.dma_start(out=pos_sb[:, j, :], in_=pos_c[:, j, :])
                if POS_MODE == "dma_add":
                    nc.gpsimd.tensor_copy(out=te[:], in_=pos_sb[:, j % Q, :])
                    nc.gpsimd.indirect_dma_start(
                        out=te[:], out_offset=None, in_=aug_w[:], in_offset=off,
                        compute_op=mybir.AluOpType.add,
                    )
                else:
                    nc.gpsimd.indirect_dma_start(
                        out=te[:], out_offset=None, in_=aug_w[:], in_offset=off,
                    )
                    add_eng = nc.gpsimd if (POS_ADD_SPLIT and j % 2) else nc.vector
                    add_eng.tensor_add(out=te[:], in0=te[:],
                                       in1=pos_sb[:, j % Q, :])

                stats = stats_p.tile([P, 2, 6], F32, tag="st")
                nc.vector.bn_stats(out=stats[:, 0, :], in_=te[:, 0:512])
                nc.vector.bn_stats(out=stats[:, 1, :], in_=te[:, 512:H])
                mv = stats_p.tile([P, 2], F32, tag="mv")
                nc.vector.bn_aggr(out=mv[:], in_=stats[:])

                std = stats_p.tile([P, 1], F32, tag="sd")
                nc.scalar.activation(
                    out=std[:], in_=mv[:, 1:2],
                    func=mybir.ActivationFunctionType.Sqrt,
                    bias=eps_sb[:], scale=1.0,
                )
                nc.vector.reciprocal(out=std[:], in_=std[:])

                o = o_p.tile([P, H], F32, tag="o")
                if NORM_MODE == "act":
                    nb = stats_p.tile([P, 1], F32, tag="nb")
                    nc.vector.tensor_scalar(
                        out=nb[:], in0=mv[:, 0:1], scalar1=std[:], scalar2=-1.0,
                        op0=mybir.AluOpType.mult, op1=mybir.AluOpType.mult,
                    )
                    nc.scalar.activation(
                        out=o[:], in_=te[:],
                        func=mybir.ActivationFunctionType.Identity,
                        bias=nb[:], scale=std[:],
                    )
                else:
                    nc.vector.tensor_scalar(
                        out=o[:], in0=te[:], scalar1=mv[:, 0:1], scalar2=std[:],
                        op0=mybir.AluOpType.subtract, op1=mybir.AluOpType.mult,
                    )
                if apply_ln:
                    nc.vector.tensor_mul(out=o[:], in0=o[:], in1=lnw_sb[:])
                    nc.vector.tensor_add(out=o[:], in0=o[:], in1=lnb_sb[:])
                nc.scalar.dma_start(out=out_d[j * P:(j + 1) * P, :], in_=o[:])
    nc.finalize()
    return nc


def _prepare_inputs(input_ids, token_type_ids, token_W, pos_W, type_W,
                    ln_w, ln_b, apply_ln):
    ids_np = np.asarray(input_ids).reshape(B, S)
    tt_np = np.asarray(token_type_ids).reshape(B, S)
    tok = np.asarray(token_W, dtype=np.float32)
    pos = np.asarray(pos_W, dtype=np.float32)
    typ = np.asarray(type_W, dtype=np.float32)

    aug = np.empty((2 * V, H), dtype=np.float32)
    aug[:V] = tok
    np.add(tok, typ[1] - typ[0], out=aug[V:])

    pos_comb = pos[:S] + typ[0]                                     # [S, H]
    pos_hw = np.ascontiguousarray(pos_comb.reshape(Q, P, H).transpose(1, 0, 2))

    ids_aug = (ids_np.astype(np.int64) + tt_np.astype(np.int64) * V).astype(np.int32)

    in_maps = []
    for c in range(N_CORES):
        idc = ids_aug[c * B_PER_CORE:(c + 1) * B_PER_CORE].reshape(N_TOK)
        m = {
            "aug_w": aug,
            "ids": np.ascontiguousarray(idc.reshape(NT, P).T),
            "pos_c": pos_hw,
        }
        if apply_ln:
            m["lnw"] = np.ascontiguousarray(
                np.asarray(ln_w, dtype=np.float32).reshape(1, H))
            m["lnb"] = np.ascontiguousarray(
                np.asarray(ln_b, dtype=np.float32).reshape(1, H))
        in_maps.append(m)
    return in_maps


def _run(input_ids, token_type_ids, token_W, pos_W, type_W, ln_w, ln_b,
         trace=False):
    lnw = np.asarray(ln_w, dtype=np.float32).reshape(-1)
    lnb = np.asarray(ln_b, dtype=np.float32).reshape(-1)
    apply_ln = not (np.all(lnw == 1.0) and np.all(lnb == 0.0))

    nc = _cache.get(apply_ln)
    if nc is None:
        nc = _cache.setdefault(apply_ln, _build(apply_ln))
    in_maps = _prepare_inputs(input_ids, token_type_ids, token_W, pos_W,
                              type_W, ln_w, ln_b, apply_ln)
    res = run_bass_kernel_spmd(nc, in_maps, list(range(N_CORES)), trace=trace)
    out = np.concatenate(
        [res.results[c]["out"].reshape(B_PER_CORE, S, H) for c in range(N_CORES)],
        axis=0,
    )
    return out, res


def kernel(input_ids, token_type_ids, token_W, pos_W, type_W, ln_w, ln_b):
    out, _ = _run(input_ids, token_type_ids, token_W, pos_W, type_W,
                  ln_w, ln_b, trace=False)
    return out


# revision 20
# speedup vs baseline: 1.1628x; 1.0050x over previous
"""BERT embedding (token/type/position gather + LayerNorm) on 8 Trainium2 cores.

Sharding: data-parallel over batch — core c handles sequences [4c, 4c+4),
i.e. 2048 tokens. Each core holds an augmented embedding table
[token_W; token_W + (type_W[1]-type_W[0])] and gathers row (id + t*V) with
indirect DMA, which folds the token-type embedding into the gather.
type_W[0] is folded into the position table on the host. The position row
is added via a GpSimd prefill + accumulate-DMA (or DVE add, configurable).
LayerNorm runs per 128-token tile with bn_stats/bn_aggr; the final
(x-mean)*rstd is applied on the scalar engine as Copy(x*rstd + (-mean*rstd)).
"""
import numpy as np

import concourse.bacc as bacc
import concourse.bass as bass
import concourse.tile as tile
from concourse import mybir
from concourse.bass_utils import run_bass_kernel_spmd

P = 128
N_CORES = 8
B, S, V, H, T = 32, 512, 30522, 1024, 2
EPS = 1e-5
B_PER_CORE = B // N_CORES       # 4 sequences per core
N_TOK = B_PER_CORE * S          # 2048 tokens per core
NT = N_TOK // P                 # 16 token tiles per core
Q = S // P                      # 4 position quarters

F32 = mybir.dt.float32
I32 = mybir.dt.int32

# POS_MODE: how pos_comb gets added to the gathered row
#   "dma_add"  — GpSimd tensor_copy prefill, gather DMA accumulates (CCE add)
#   "dve_add"  — plain gather, DVE tensor_add afterwards
POS_MODE = "dve_add"
# NORM_MODE: "act" = Copy(x*rstd + nb) on scalar engine; "dve" = tensor_scalar
NORM_MODE = "act"
BUFS_TE = 16
BUFS_O = 6
POS_ADD_SPLIT = False

_cache: dict = {}


def _build(apply_ln: bool):
    nc = bacc.Bacc(None, target_bir_lowering=False)
    aug_w = nc.declare_dram_parameter("aug_w", [2 * V, H], F32, isOutput=False)
    ids = nc.declare_dram_parameter("ids", [P, NT], I32, isOutput=False)
    pos_c = nc.declare_dram_parameter("pos_c", [P, Q, H], F32, isOutput=False)
    if apply_ln:
        lnw = nc.declare_dram_parameter("lnw", [1, H], F32, isOutput=False)
        lnb = nc.declare_dram_parameter("lnb", [1, H], F32, isOutput=False)
    out_d = nc.declare_dram_parameter("out", [N_TOK, H], F32, isOutput=True)

    with tile.TileContext(nc) as tc:
        with (
            tc.tile_pool(name="singles", bufs=1) as singles,
            tc.tile_pool(name="te_p", bufs=BUFS_TE) as te_p,
            tc.tile_pool(name="o_p", bufs=BUFS_O) as o_p,
            tc.tile_pool(name="stats", bufs=8) as stats_p,
        ):
            ids_sb = singles.tile([P, NT], I32)
            nc.sync.dma_start(out=ids_sb[:], in_=ids[:])
            pos_sb = singles.tile([P, Q, H], F32)
            nc.sync.dma_start(out=pos_sb[:, 0, :], in_=pos_c[:, 0, :])
            eps_sb = singles.tile([P, 1], F32)
            nc.vector.memset(eps_sb[:], EPS)
            if apply_ln:
                lnw_sb = singles.tile([P, H], F32)
                nc.gpsimd.dma_start(out=lnw_sb[:], in_=lnw[:].to_broadcast([P, H]))
                lnb_sb = singles.tile([P, H], F32)
                nc.gpsimd.dma_start(out=lnb_sb[:], in_=lnb[:].to_broadcast([P, H]))

            for j in range(NT):
                te = te_p.tile([P, H], F32, tag="te")
                off = bass.IndirectOffsetOnAxis(ap=ids_sb[:, j:j + 1], axis=0)
                if POS_MODE == "dma_add":
                    nc.gpsimd.tensor_copy(out=te[:], in_=pos_sb[:, j % Q, :])
                    gather = nc.gpsimd.indirect_dma_start(
                        out=te[:], out_offset=None, in_=aug_w[:], in_offset=off,
                        compute_op=mybir.AluOpType.add,
                    )
                else:
                    gather = nc.gpsimd.indirect_dma_start(
                        out=te[:], out_offset=None, in_=aug_w[:], in_offset=off,
                    )
                if 1 <= j <= Q - 1:
                    # pos quarter j loads only after gather j's descgen so the
                    # first gathers' data isn't queued behind 1.5MB of pos
                    ld = nc.sync.dma_start(out=pos_sb[:, j, :],
                                           in_=pos_c[:, j, :])
                    tile.add_dep_helper(
                        gather.ins, ld.ins, sync=True,
                        reason="defer pos quarter past early gathers",
                    )
                if POS_MODE != "dma_add":
                    add_eng = nc.gpsimd if (POS_ADD_SPLIT and j % 2) else nc.vector
                    add_eng.tensor_add(out=te[:], in0=te[:],
                                       in1=pos_sb[:, j % Q, :])

                stats = stats_p.tile([P, 2, 6], F32, tag="st")
                nc.vector.bn_stats(out=stats[:, 0, :], in_=te[:, 0:512])
                nc.vector.bn_stats(out=stats[:, 1, :], in_=te[:, 512:H])
                mv = stats_p.tile([P, 2], F32, tag="mv")
                nc.vector.bn_aggr(out=mv[:], in_=stats[:])

                std = stats_p.tile([P, 1], F32, tag="sd")
                nc.scalar.activation(
                    out=std[:], in_=mv[:, 1:2],
                    func=mybir.ActivationFunctionType.Sqrt,
                    bias=eps_sb[:], scale=1.0,
                )
                nc.vector.reciprocal(out=std[:], in_=std[:])

                o = o_p.tile([P, H], F32, tag="o")
                norm_mode = "dve" if j >= NT - 4 else NORM_MODE
                if norm_mode == "act":
                    nb = stats_p.tile([P, 1], F32, tag="nb")
                    nc.vector.tensor_scalar(
                        out=nb[:], in0=mv[:, 0:1], scalar1=std[:], scalar2=-1.0,
                        op0=mybir.AluOpType.mult, op1=mybir.AluOpType.mult,
                    )
                    nc.scalar.activation(
                        out=o[:], in_=te[:],
                        func=mybir.ActivationFunctionType.Identity,
                        bias=nb[:], scale=std[:],
                    )
                else:
                    nc.vector.tensor_scalar(
                        out=o[:], in0=te[:], scalar1=mv[:, 0:1], scalar2=std[:],
                        op0=mybir.AluOpType.subtract, op1=mybir.AluOpType.mult,
                    )
                if apply_ln:
                    nc.vector.tensor_mul(out=o[:], in0=o[:], in1=lnw_sb[:])
                    nc.vector.tensor_add(out=o[:], in0=o[:], in1=lnb_sb[:])
                nc.scalar.dma_start(out=out_d[j * P:(j + 1) * P, :], in_=o[:])
    nc.finalize()
    return nc


def _prepare_inputs(input_ids, token_type_ids, token_W, pos_W, type_W,
                    ln_w, ln_b, apply_ln):
    ids_np = np.asarray(input_ids).reshape(B, S)
    tt_np = np.asarray(token_type_ids).reshape(B, S)
    tok = np.asarray(token_W, dtype=np.float32)
    pos = np.asarray(pos_W, dtype=np.float32)
    typ = np.asarray(type_W, dtype=np.float32)

    aug = np.empty((2 * V, H), dtype=np.float32)
    aug[:V] = tok
    np.add(tok, typ[1] - typ[0], out=aug[V:])

    pos_comb = pos[:S] + typ[0]                                     # [S, H]
    pos_hw = np.ascontiguousarray(pos_comb.reshape(Q, P, H).transpose(1, 0, 2))

    ids_aug = (ids_np.astype(np.int64) + tt_np.astype(np.int64) * V).astype(np.int32)

    in_maps = []
    for c in range(N_CORES):
        idc = ids_aug[c * B_PER_CORE:(c + 1) * B_PER_CORE].reshape(N_TOK)
        m = {
            "aug_w": aug,
            "ids": np.ascontiguousarray(idc.reshape(NT, P).T),
            "pos_c": pos_hw,
        }
        if apply_ln:
            m["lnw"] = np.ascontiguousarray(
                np.asarray(ln_w, dtype=np.float32).reshape(1, H))
            m["lnb"] = np.ascontiguousarray(
                np.asarray(ln_b, dtype=np.float32).reshape(1, H))
        in_maps.append(m)
    return in_maps


def _run(input_ids, token_type_ids, token_W, pos_W, type_W, ln_w, ln_b,
         trace=False):
    lnw = np.asarray(ln_w, dtype=np.float32).reshape(-1)
    lnb = np.asarray(ln_b, dtype=np.float32).reshape(-1)
    apply_ln = not (np.all(lnw == 1.0) and np.all(lnb == 0.0))

    nc = _cache.get(apply_ln)
    if nc is None:
        nc = _cache.setdefault(apply_ln, _build(apply_ln))
    in_maps = _prepare_inputs(input_ids, token_type_ids, token_W, pos_W,
                              type_W, ln_w, ln_b, apply_ln)
    res = run_bass_kernel_spmd(nc, in_maps, list(range(N_CORES)), trace=trace)
    out = np.concatenate(
        [res.results[c]["out"].reshape(B_PER_CORE, S, H) for c in range(N_CORES)],
        axis=0,
    )
    return out, res


def kernel(input_ids, token_type_ids, token_W, pos_W, type_W, ln_w, ln_b):
    out, _ = _run(input_ids, token_type_ids, token_W, pos_W, type_W,
                  ln_w, ln_b, trace=False)
    return out
